# revision 1
# baseline (speedup 1.0000x reference)
"""HardBatchMiningTripletLoss on 8 Trainium2 NeuronCores (Bass/Tile).

Math: dist(i,j) = sqrt(clip(sqrt(clip(d2,1e-24)),1e-12)) = clip(d2)^(1/4) is a
monotone map of d2 = sq_i + sq_j - 2*x_i.x_j, so the row-wise hard mining
(min over same-label, max over diff-label) can run on d2-level values and the
quartic root is applied only to the per-row selected scalars on the host.
sq_i is constant per row, so it commutes with the row reductions and is also
applied on host. The device computes, per row i:
    rmin_i = min_{j in window} (-2*G_ij + sq_j - 4096*eq_ij)   -> pos_min - 4096
    rmax_i = max_{j}           (-2*G_ij + sq_j - 4096*eq_ij)   -> neg_max
where eq_ij = [label_i == label_j]. Rows+columns are pre-sorted by label and
each core's columns are rotated so that, for row-tile rt, all same-label
columns of its 128 rows fall in the static window [rt*128, rt*128+256): the
penalty mask is only needed there, everything outside is pure negatives.

Sharding: data parallel over rows - core c handles sorted rows
[c*1024, (c+1)*1024) against all 8192 columns (full inputs re-read per core).
"""

import os

import numpy as np

B = 8192          # batch
D = 256           # feature dim
NCORES = 8
M = B // NCORES   # rows per core
P = 128           # partitions
KT = D // P       # k-chunks per matmul (2)
MT = M // P       # row-tiles per core (8)
WIN = 256         # label window columns (requires max class size <= 64)
PAD = 64          # rotation back-offset
TW = M - P + WIN  # window columns union (1152)
BIG = 4096.0      # additive mask penalty; > max d2 (~1000)
NMM = 512         # matmul moving free dim
PS_CH = 2048      # psum tile columns (4 banks)
MARGIN = 0.3

_CACHE = {}


def _emit(tc, outs, ins):
    """Tile kernel body. ins/outs: dicts of DRAM APs."""
    import concourse.bass as bass
    from concourse import mybir

    nc = tc.nc
    f32 = mybir.dt.float32
    bf16 = mybir.dt.bfloat16
    f16 = mybir.dt.float16
    Alu = mybir.AluOpType
    Act = mybir.ActivationFunctionType

    rhs_d, lhsT_d, sqc_d, tw_d, trows_d = (
        ins["rhs"], ins["lhsT"], ins["sqc"], ins["tw"], ins["trows"])
    stats_d = outs["stats"]

    with (
        tc.tile_pool(name="singles", bufs=1) as singles,
        tc.tile_pool(name="vpool", bufs=2) as vpool,
        tc.tile_pool(name="wpool", bufs=2) as wpool,
        tc.tile_pool(name="accpool", bufs=6) as accpool,
        tc.tile_pool(name="psum", bufs=2, space="PSUM") as pspool,
    ):
        # --- one-time loads -------------------------------------------------
        rhs_sb = []
        lhsT_sb = []
        for k in range(KT):
            rt_t = singles.tile([P, B], bf16, tag=f"rhs{k}")
            nc.sync.dma_start(out=rt_t, in_=rhs_d[k])
            rhs_sb.append(rt_t)
            lt_t = singles.tile([P, M], bf16, tag=f"lhsT{k}")
            nc.sync.dma_start(out=lt_t, in_=lhsT_d[k])
            lhsT_sb.append(lt_t)
        # sq of columns on partition 0 (rhs row for the K=1 ones matmul)
        sqc_sb = singles.tile([1, B], bf16, tag="sqc")
        nc.sync.dma_start(out=sqc_sb, in_=sqc_d)
        ones_sb = singles.tile([1, P], bf16, tag="ones")
        nc.vector.memset(ones_sb, 1.0)
        twb_raw = singles.tile([P, TW], f16, tag="twb_raw")
        nc.gpsimd.dma_start(
            out=twb_raw, in_=bass.AP(tw_d.tensor, tw_d.offset, [[0, P], [1, TW]]))
        twb = singles.tile([P, TW], f16, tag="twb")
        nc.vector.tensor_copy(twb, twb_raw)
        trows_raw = singles.tile([P, MT], f32, tag="trows_raw")
        nc.sync.dma_start(out=trows_raw, in_=trows_d)
        # stage via VE so TensorScalarPtr (single sync-wait slot) only ever
        # depends on same-engine producers
        trows = singles.tile([P, MT], f32, tag="trows")
        nc.vector.tensor_copy(trows, trows_raw)
        stats_sb = singles.tile([P, 2 * MT], f32, tag="stats")

        # --- main loop over row-tiles --------------------------------------
        for rt in range(MT):
            # v0 = -2*G + sq_j  (sq_j accumulated on PE via ones-row matmul)
            v0 = vpool.tile([P, B], bf16, tag="v0")
            for g in range(B // PS_CH):
                ps = pspool.tile([P, PS_CH], f32, tag="ps")
                for k in range(KT):
                    for n in range(PS_CH // NMM):
                        col = g * PS_CH + n * NMM
                        nc.tensor.matmul(
                            ps[:, n * NMM:(n + 1) * NMM],
                            lhsT_sb[k][:, rt * P:(rt + 1) * P],
                            rhs_sb[k][:, col:col + NMM],
                            start=(k == 0), stop=False)
                for n in range(PS_CH // NMM):
                    col = g * PS_CH + n * NMM
                    nc.tensor.matmul(
                        ps[:, n * NMM:(n + 1) * NMM],
                        ones_sb,
                        sqc_sb[:, col:col + NMM],
                        start=False, stop=True)
                nc.scalar.activation(
                    out=v0[:, g * PS_CH:(g + 1) * PS_CH], in_=ps,
                    func=Act.Copy)

            w0 = rt * P             # window start
            w1 = rt * P + WIN       # window end
            # outer regions [0,w0) and [w1,B) hold only negatives
            accs = []
            for lo, hi in ((0, w0), (w1, B)):
                if lo >= hi:
                    continue
                nacc = accpool.tile([P, 1], f32, tag="acc")
                nc.vector.tensor_reduce(
                    out=nacc, in_=v0[:, lo:hi], axis=mybir.AxisListType.X,
                    op=Alu.max)
                accs.append(nacc)
            # window: v + (-BIG)*eq -> positives sink below all negatives
            eqw = wpool.tile([P, WIN], bf16, tag="eqw")
            nc.vector.tensor_scalar(
                out=eqw, in0=twb[:, w0:w1],
                scalar1=trows[:, rt:rt + 1], scalar2=-BIG,
                op0=Alu.is_equal, op1=Alu.mult)
            win1 = wpool.tile([P, WIN], bf16, tag="win1")
            nc.vector.tensor_add(win1, v0[:, w0:w1], eqw)
            # pos_min - BIG
            nc.vector.tensor_reduce(
                out=stats_sb[:, 2 * rt:2 * rt + 1], in_=win1,
                axis=mybir.AxisListType.X, op=Alu.min)
            # window negatives still at true value -> max over win1
            wacc = accpool.tile([P, 1], f32, tag="acc")
            nc.vector.tensor_reduce(
                out=wacc, in_=win1, axis=mybir.AxisListType.X, op=Alu.max)
            accs.append(wacc)
            # combine outer + window neg maxima
            comb = accs[0]
            for a in accs[1:]:
                ncomb = accpool.tile([P, 1], f32, tag="acc")
                nc.vector.tensor_max(ncomb, comb, a)
                comb = ncomb
            nc.vector.tensor_copy(stats_sb[:, 2 * rt + 1:2 * rt + 2], comb)

        nc.sync.dma_start(out=stats_d, in_=stats_sb)


def _build():
    import concourse.tile as tile
    from concourse import bacc, mybir

    nc = bacc.Bacc("TRN2", target_bir_lowering=False, debug=False,
                   num_devices=NCORES)
    f32, bf16, f16 = mybir.dt.float32, mybir.dt.bfloat16, mybir.dt.float16
    ins = {
        "rhs": nc.dram_tensor("rhs", [KT, P, B], bf16, kind="ExternalInput").ap(),
        "lhsT": nc.dram_tensor("lhsT", [KT, P, M], bf16, kind="ExternalInput").ap(),
        "sqc": nc.dram_tensor("sqc", [1, B], bf16, kind="ExternalInput").ap(),
        "tw": nc.dram_tensor("tw", [1, TW], f16, kind="ExternalInput").ap(),
        "trows": nc.dram_tensor("trows", [P, MT], f32, kind="ExternalInput").ap(),
    }
    outs = {
        "stats": nc.dram_tensor("stats", [P, 2 * MT], f32,
                                kind="ExternalOutput").ap(),
    }
    with tile.TileContext(nc) as tc:
        _emit(tc, outs, ins)
    nc.compile()  # bacc passes incl. generate_event_semaphores (1-wait limit)
    return nc


def _get_nc():
    if "nc" not in _CACHE:
        _CACHE["nc"] = _build()
    return _CACHE["nc"]


def _host_prep(x, t):
    """Sort by label, build per-core input maps."""
    import ml_dtypes

    perm = np.argsort(t, kind="stable")
    xs = np.ascontiguousarray(x[perm])          # [B, D] fp32, label-sorted
    ts = t[perm].astype(np.int64)
    sq = np.einsum("ij,ij->i", xs, xs, dtype=np.float32)  # [B]

    in_maps = []
    for c in range(NCORES):
        rows = slice(c * M, (c + 1) * M)
        # local col k <-> sorted col (c*M - PAD + k) mod B
        rot = (np.arange(B) + c * M - PAD) % B
        rhs = xs[rot].T.reshape(KT, P, B)                       # [2,128,B]
        lhsT = (-2.0 * xs[rows]).T.reshape(KT, P, M)            # [2,128,M]
        sqc = sq[rot][None, :]                                  # [1,B]
        tw = ts[rot[:TW]][None, :]                              # [1,TW]
        trows = ts[rows].reshape(MT, P).T                       # [128,MT]
        in_maps.append({
            "rhs": rhs.astype(ml_dtypes.bfloat16),
            "lhsT": lhsT.astype(ml_dtypes.bfloat16),
            "sqc": sqc.astype(ml_dtypes.bfloat16),
            "tw": tw.astype(np.float16),
            "trows": trows.astype(np.float32),
        })
    return perm, xs, ts, sq, in_maps


def _final_loss(pos_min_d2, neg_max_d2):
    """Mirror the reference epilogue in fp32."""
    def quartic(d2):
        d = np.sqrt(np.clip(d2.astype(np.float32), np.float32(1e-24), None))
        return np.sqrt(np.clip(d, np.float32(1e-12), None))
    d_pos = quartic(pos_min_d2)
    d_neg = quartic(neg_max_d2)
    per_row = np.maximum(d_pos - d_neg + np.float32(MARGIN), np.float32(0.0))
    return np.array(np.mean(per_row), dtype=np.float32)


def _numpy_fallback(x, t):
    sq = np.einsum("ij,ij->i", x, x, dtype=np.float32)
    d2 = sq[:, None] + sq[None, :] - 2.0 * (x @ x.T)
    d = np.sqrt(np.clip(d2, np.float32(1e-24), None))
    dist = np.sqrt(np.clip(d, np.float32(1e-12), None))
    valid = t != -1
    same = t[:, None] == t[None, :]
    pos_mask = same & valid[None, :]
    neg_mask = (~same) & valid[None, :]
    inf = np.float32(np.inf)
    pos_count = pos_mask.sum(1)
    pos_min = np.where(pos_mask, dist, inf).min(1)
    pos_max = np.where(pos_mask, dist, -inf).max(1)
    d_pos = np.where(pos_count > 1, pos_min, pos_max)
    neg_count = neg_mask.sum(1)
    neg_max = np.where(neg_mask, dist, -inf).max(1)
    notneg_min = np.where(~neg_mask, dist, inf).min(1)
    d_neg = np.where(neg_count > 0, neg_max, notneg_min)
    loss = np.mean(np.maximum(d_pos - d_neg + np.float32(MARGIN), 0.0))
    return np.array(loss, dtype=np.float32)


def kernel(inputs, targets):
    from concourse.bass_utils import run_bass_kernel_spmd

    x = np.asarray(inputs, dtype=np.float32)
    t = np.asarray(targets).astype(np.int64)
    assert x.shape == (B, D) and t.shape == (B,)

    counts = np.bincount(t[t >= 0], minlength=1) if (t >= 0).any() else np.array([0])
    if (t == -1).any() or counts.max() > PAD or counts.max() >= B:
        # degenerate label patterns the device layout doesn't cover
        return _numpy_fallback(x, t)

    perm, xs, ts, sq, in_maps = _host_prep(x, t)
    nc = _get_nc()
    res = run_bass_kernel_spmd(nc, in_maps, core_ids=list(range(NCORES)))
    _CACHE["last_run"] = res

    pos_min_d2 = np.empty(B, np.float32)
    neg_max_d2 = np.empty(B, np.float32)
    for c in range(NCORES):
        st = res.results[c]["stats"].reshape(P, MT, 2)   # [p, rt, 2]
        rows = c * M + np.arange(MT) * P + np.arange(P)[:, None]  # [p, rt]
        pos_min_d2[rows] = st[:, :, 0] + np.float32(BIG) + sq[rows]
        neg_max_d2[rows] = st[:, :, 1] + sq[rows]
    # rows are in sorted order; loss is a mean so order does not matter
    return _final_loss(pos_min_d2, neg_max_d2)

